# revision 20
# baseline (speedup 1.0000x reference)
"""Multi-head attention (B=2, S=2048, D=1024, H=16) on 8 TRN2 NeuronCores.

Sharding: core c handles batch c//4 and head-group c%4 (4 heads each).
Host pre-transposes inputs/weights to d-major bf16; each core computes its
4 heads' projections, causal attention, and a partial (row-parallel) dense
output [S, D] which the host sums across the 4 cores of each batch.

Attention math: scores are computed transposed ([k, q] layout, q on the
free dim) so no on-chip transposes are ever needed.  Masking is applied as
a multiplicative factor F = exp(-1e9*m/8) on the exp'd scores (exact for
0/1 masks, correct in general); fully-masked 128x512 tiles are skipped at
emit time based on the actual mask contents.  Softmax row sums come for
free from a ones-column appended to the V tiles in the AV matmul; the
reciprocal is broadcast across partitions via a small DRAM bounce.
"""

import numpy as np
import ml_dtypes
from contextlib import ExitStack

import concourse.bass as bass
import concourse.tile as tile
from concourse import bacc, mybir
from concourse.bass_utils import run_bass_kernel_spmd

BF16 = mybir.dt.bfloat16
F32 = mybir.dt.float32
NPBF16 = ml_dtypes.bfloat16

D_MODEL = 1024
NH = 16
DEPTH = 64
B = 2
S = 2048
N_CORES = 8
GROUPS = 4              # head-groups (tensor parallel dimension)
HPG = NH // GROUPS      # 4 heads per core
OG = HPG * DEPTH        # 256 projection output cols per core
QC = 512                # q chunk (matmul free dim)
NQC = S // QC           # 4
KT = 128                # k tile (psum partition dim)
NKT = S // KT           # 16
DK = D_MODEL // 128     # 8 contraction tiles of 128
SC = 512                # projection s chunk
NSC = S // SC           # 4
EGRP = 2                # k-tiles per exp group (psum group tile)

TRACE = False
TRACE_KW = {}
LAST_RESULT = None
DEBUG = False
_CACHE = {}


def _chunk(lst, n):
    return [lst[i : i + n] for i in range(0, len(lst), n)]


def _build(ktiles, mults, n_uniq):
    """Emit the bass program. ktiles[j] = list of computed k-tiles for
    q-chunk j; mults[j][t] = mask-factor tile id (or absent)."""
    nc = bacc.Bacc(
        "TRN2", target_bir_lowering=False, debug=False, num_devices=N_CORES
    )
    xq = nc.dram_tensor("xq", [128, DK, S], BF16, kind="ExternalInput").ap()
    xk = nc.dram_tensor("xk", [128, DK, S], BF16, kind="ExternalInput").ap()
    xv = nc.dram_tensor("xv", [128, DK, S], BF16, kind="ExternalInput").ap()
    wq = nc.dram_tensor("wq", [128, DK, OG], BF16, kind="ExternalInput").ap()
    wk = nc.dram_tensor("wk", [128, DK, OG], BF16, kind="ExternalInput").ap()
    wv = nc.dram_tensor("wv", [128, DK, OG], BF16, kind="ExternalInput").ap()
    wd = nc.dram_tensor("wd", [128, 2, D_MODEL], BF16, kind="ExternalInput").ap()
    qb = nc.dram_tensor("qb", [128, 2], F32, kind="ExternalInput").ap()
    kb = nc.dram_tensor("kb", [128, 2], F32, kind="ExternalInput").ap()
    mk = nc.dram_tensor("mk", [128, n_uniq, QC], BF16, kind="ExternalInput").ap()
    outp = nc.dram_tensor("outp", [S, D_MODEL], F32, kind="ExternalOutput").ap()

    Ident = mybir.ActivationFunctionType.Identity
    Exp = mybir.ActivationFunctionType.Exp

    with tile.TileContext(nc) as tc, ExitStack() as ctx:
        singles = ctx.enter_context(tc.tile_pool(name="singles", bufs=1))
        stage = ctx.enter_context(tc.tile_pool(name="stage", bufs=3))
        exps = ctx.enter_context(tc.tile_pool(name="exps", bufs=3))
        small = ctx.enter_context(tc.tile_pool(name="small", bufs=2))
        bcastp = ctx.enter_context(tc.tile_pool(name="bcastp", bufs=3))
        dram = ctx.enter_context(tc.tile_pool(name="dram", bufs=2, space="DRAM"))
        pp = ctx.enter_context(tc.tile_pool(name="pp", bufs=2, space="PSUM"))
        psc = ctx.enter_context(tc.tile_pool(name="psc", bufs=2, space="PSUM"))
        pav = ctx.enter_context(tc.tile_pool(name="pav", bufs=2, space="PSUM"))

        wq_sb = singles.tile([128, DK, OG], BF16)
        nc.sync.dma_start(wq_sb[:], wq)
        wk_sb = singles.tile([128, DK, OG], BF16)
        nc.sync.dma_start(wk_sb[:], wk)
        wv_sb = singles.tile([128, DK, OG], BF16)
        nc.sync.dma_start(wv_sb[:], wv)
        wd_sb = singles.tile([128, 2, D_MODEL], BF16)
        nc.sync.dma_start(wd_sb[:], wd)
        mk_sb = singles.tile([128, n_uniq, QC], BF16)
        nc.sync.dma_start(mk_sb[:], mk)
        qb_sb = singles.tile([128, 2], F32)
        nc.sync.dma_start(qb_sb[:], qb)
        kb_sb = singles.tile([128, 2], F32)
        nc.sync.dma_start(kb_sb[:], kb)

        # o = co*128 + p layouts
        qt = singles.tile([128, 2, S], BF16)
        kt_ = singles.tile([128, 2, S], BF16)
        # [p=k%128, ktile, head, 64 d cols + ones col]
        vh1 = singles.tile([128, NKT, HPG, 65], BF16)
        avf = singles.tile([128, 2, S], F32)    # unnormalized av^T
        avb = singles.tile([128, 2, S], BF16)   # normalized av^T

        nc.vector.memset(vh1[:, :, :, 64:65], 1.0)

        # ---- q/k projections: out[o, s] = w[:, o].T @ x[:, s] ----
        for x_ap, w_sb, b_sb, dst in (
            (xq, wq_sb, qb_sb, qt),
            (xk, wk_sb, kb_sb, kt_),
        ):
            for sc in range(NSC):
                xt = stage.tile([128, DK, SC], BF16, tag="xstage")
                nc.sync.dma_start(xt[:], x_ap[:, :, sc * SC : (sc + 1) * SC])
                for oc in range(2):
                    ps = pp.tile([128, SC], F32, tag="pp")
                    for dk in range(DK):
                        nc.tensor.matmul(
                            ps[:],
                            lhsT=w_sb[:, dk, oc * 128 : (oc + 1) * 128],
                            rhs=xt[:, dk, :],
                            start=(dk == 0),
                            stop=(dk == DK - 1),
                        )
                    nc.scalar.activation(
                        out=dst[:, oc, sc * SC : (sc + 1) * SC],
                        in_=ps[:],
                        func=Ident,
                        bias=b_sb[:, oc : oc + 1],
                        scale=1.0,
                    )

        # ---- v projection: out[s, o] = x[:, s].T @ w[:, o] ----
        for st in range(NKT):
            xt = stage.tile([128, DK, KT], BF16, tag="vstage")
            nc.sync.dma_start(xt[:], xv[:, :, st * KT : (st + 1) * KT])
            ps = pp.tile([128, SC], F32, tag="pp")
            for dk in range(DK):
                nc.tensor.matmul(
                    ps[:, :OG],
                    lhsT=xt[:, dk, :],
                    rhs=wv_sb[:, dk, :],
                    start=(dk == 0),
                    stop=(dk == DK - 1),
                )
            nc.vector.tensor_copy(
                out=vh1[:, st, :, 0:64],
                in_=ps[:, :OG].rearrange("p (h d) -> p h d", d=DEPTH),
            )

        # ---- attention, per local head ----
        dbg_rec = (
            nc.dram_tensor("dbg_rec", [HPG, NQC, QC], F32, kind="ExternalOutput").ap()
            if DEBUG
            else None
        )
        dbg_den = (
            nc.dram_tensor("dbg_den", [HPG, NQC, QC], F32, kind="ExternalOutput").ap()
            if DEBUG
            else None
        )
        dbg_bc = (
            nc.dram_tensor(
                "dbg_bc", [HPG, NQC, 128, QC], F32, kind="ExternalOutput"
            ).ap()
            if DEBUG
            else None
        )
        for h in range(HPG):
            odd = h % 2
            pb = odd * 64
            ch = h // 2
            rec_stage = small.tile([128, NQC, QC], F32, tag="recst")
            den_stage = small.tile([128, NQC, QC], F32, tag="denst")
            for j in range(NQC):
                tiles = ktiles[j]
                first, last = tiles[0], tiles[-1]
                ps_av = pav.tile([128, QC], F32, tag="pav")
                for grp in _chunk(tiles, EGRP):
                    ps_g = psc.tile([128, EGRP, QC], F32, tag="psc")
                    for r, t in enumerate(grp):
                        nc.tensor.matmul(
                            ps_g[:, r, :],
                            lhsT=kt_[pb : pb + 64, ch, t * KT : (t + 1) * KT],
                            rhs=qt[pb : pb + 64, ch, j * QC : (j + 1) * QC],
                            start=True,
                            stop=True,
                        )
                    ex = exps.tile([128, EGRP, QC], BF16, tag="exps")
                    nc.scalar.activation(
                        out=ex[:, : len(grp), :],
                        in_=ps_g[:, : len(grp), :],
                        func=Exp,
                        scale=0.125,
                    )
                    for r, t in enumerate(grp):
                        uid = mults[j].get(t)
                        if uid is not None:
                            nc.gpsimd.tensor_mul(
                                ex[:, r, :], ex[:, r, :], mk_sb[:, uid, :]
                            )
                    for r, t in enumerate(grp):
                        nc.tensor.matmul(
                            ps_av[0:65, :],
                            lhsT=vh1[:, t, h, :],
                            rhs=ex[:, r, :],
                            start=(t == first),
                            stop=(t == last),
                        )
                if odd:
                    # engines can't shift partitions; bounce via SBUF + DMA
                    tmp = bcastp.tile([64, QC], F32, tag="avtmp")
                    nc.vector.tensor_copy(out=tmp[:], in_=ps_av[0:64, :])
                    nc.sync.dma_start(
                        avf[64:128, ch, j * QC : (j + 1) * QC], tmp[:]
                    )
                else:
                    nc.vector.tensor_copy(
                        out=avf[0:64, ch, j * QC : (j + 1) * QC],
                        in_=ps_av[0:64, :],
                    )
                nc.vector.tensor_copy(
                    out=den_stage[64:65, j, :], in_=ps_av[64:65, :]
                )
            # custom DVE recip op requires base partition 0 -> DMA-shift rows
            den0 = small.tile([NQC, QC], F32, tag="den0")
            nc.sync.dma_start(den0[:], den_stage[64:65, :, :])
            rec0 = small.tile([NQC, QC], F32, tag="rec0")
            nc.vector.reciprocal_approx_fast(rec0[:], den0[:])
            rec_t = dram.tile([NQC, QC], F32, tag="rec")
            nc.sync.dma_start(rec_t[:], rec0[:])
            if DEBUG:
                nc.sync.dma_start(dbg_rec[h : h + 1, :, :], rec0[:])
                nc.sync.dma_start(dbg_den[h : h + 1, :, :], den_stage[64:65, :, :])
            for j in range(NQC):
                bc = bcastp.tile([128, QC], F32, tag="bc")
                nc.sync.dma_start(
                    bc[pb : pb + 64, :],
                    rec_t[j : j + 1, :].to_broadcast([64, QC]),
                )
                if DEBUG:
                    nc.sync.dma_start(dbg_bc[h, j, pb : pb + 64, :], bc[pb : pb + 64, :])
                nc.vector.tensor_mul(
                    avb[pb : pb + 64, ch, j * QC : (j + 1) * QC],
                    avf[pb : pb + 64, ch, j * QC : (j + 1) * QC],
                    bc[pb : pb + 64, :],
                )

        # ---- dense (row-parallel partial): out[s, :] = av^T.T @ wd ----
        for st in range(NKT):
            for oc in range(2):
                ps = pp.tile([128, SC], F32, tag="pp")
                for co in range(2):
                    nc.tensor.matmul(
                        ps[:],
                        lhsT=avb[:, co, st * 128 : (st + 1) * 128],
                        rhs=wd_sb[:, co, oc * 512 : (oc + 1) * 512],
                        start=(co == 0),
                        stop=(co == 1),
                    )
                ot = stage.tile([128, SC], F32, tag="ostage")
                nc.vector.tensor_copy(out=ot[:], in_=ps[:])
                nc.sync.dma_start(
                    outp[st * 128 : (st + 1) * 128, oc * 512 : (oc + 1) * 512],
                    ot[:],
                )

        if DEBUG:
            for name, t in (
                ("dbg_qt", qt),
                ("dbg_kt", kt_),
                ("dbg_vh1", vh1),
                ("dbg_avf", avf),
                ("dbg_avb", avb),
            ):
                dt_ = nc.dram_tensor(
                    name, list(t.shape), t.dtype, kind="ExternalOutput"
                ).ap()
                nc.sync.dma_start(dt_, t[:])

    nc.compile()
    return nc


def _classify_mask(mask):
    """Classify 128(k) x 512(q) score tiles from the actual mask contents.

    Returns (ktiles, mults, mk_arr):
      ktiles[j]: k-tile indices to compute for q-chunk j
      mults[j]: {t: unique factor tile id}
      mk_arr: [128, NU, 512] bf16 multiplicative factors exp(-1e9*m/8)
    """
    m2 = np.asarray(mask, dtype=np.float32).reshape(S, S)
    F = np.exp(m2 * np.float32(-1.25e8))  # exp(-1e9*m/8); 0/1 masks -> 0/1
    if (F.max(axis=1) == 0.0).any():
        raise RuntimeError("mask has fully-masked rows; unsupported")
    blocks = F.reshape(NKT, 128, NKT, 128)  # [qi, qr, t, kr]
    kept = (blocks == 1.0).all(axis=(1, 3))  # [qi, t]
    skip = (blocks == 0.0).all(axis=(1, 3))

    ktiles = []
    mults = []
    uniq = {}
    mk_tiles = []
    for j in range(NQC):
        qis = range(j * (QC // 128), (j + 1) * (QC // 128))
        tl = []
        mu = {}
        for t in range(NKT):
            if all(skip[qi, t] for qi in qis):
                continue
            tl.append(t)
            if all(kept[qi, t] for qi in qis):
                continue
            fb = np.ascontiguousarray(
                F[j * QC : (j + 1) * QC, t * KT : (t + 1) * KT].T
            ).astype(NPBF16)
            key = fb.tobytes()
            if key not in uniq:
                uniq[key] = len(mk_tiles)
                mk_tiles.append(fb)
            mu[t] = uniq[key]
        if not tl:
            raise RuntimeError("q-chunk with no kept k-tiles; unsupported")
        ktiles.append(tl)
        mults.append(mu)
    if not mk_tiles:
        mk_tiles.append(np.ones((128, QC), dtype=NPBF16))
    mk_arr = np.ascontiguousarray(np.stack(mk_tiles, axis=0).transpose(1, 0, 2))
    return ktiles, mults, mk_arr


def _xt_prep(x):
    """[S, D] f32 -> [128, DK, S] bf16, d-major (d = do*128 + di)."""
    xt = x.T.astype(NPBF16)  # [D, S]
    return np.ascontiguousarray(xt.reshape(DK, 128, S).transpose(1, 0, 2))


def kernel(v, k, q, mask, wq_w, wq_b, wk_w, wk_b, wv_w, wv_b, dense_w, dense_b):
    global LAST_RESULT
    v = np.asarray(v, dtype=np.float32)
    k = np.asarray(k, dtype=np.float32)
    q = np.asarray(q, dtype=np.float32)
    mask = np.asarray(mask, dtype=np.float32)
    wq_w = np.asarray(wq_w, dtype=np.float32)
    wk_w = np.asarray(wk_w, dtype=np.float32)
    wv_w = np.asarray(wv_w, dtype=np.float32)
    dense_w = np.asarray(dense_w, dtype=np.float32)
    wq_b = np.asarray(wq_b, dtype=np.float32)
    wk_b = np.asarray(wk_b, dtype=np.float32)
    wv_b = np.asarray(wv_b, dtype=np.float32)
    dense_b = np.asarray(dense_b, dtype=np.float32)

    ktiles, mults, mk_arr = _classify_mask(mask)
    key = (
        tuple(tuple(t) for t in ktiles),
        tuple(tuple(sorted(m.items())) for m in mults),
        mk_arr.shape[1],
    )
    if key not in _CACHE:
        _CACHE[key] = _build(ktiles, mults, mk_arr.shape[1])
    nc = _CACHE[key]

    # per-batch inputs (shared by the 4 cores of each batch)
    xq_b = [_xt_prep(q[b]) for b in range(B)]
    xk_b = [_xt_prep(k[b]) for b in range(B)]
    xv_b = [_xt_prep(v[b]) for b in range(B)]

    # per-group weights
    def wslice(w, g):
        ws = w[g * OG : (g + 1) * OG, :].T.astype(NPBF16)  # [D, OG]
        return np.ascontiguousarray(ws.reshape(DK, 128, OG).transpose(1, 0, 2))

    def bslice(b_, g):
        return np.ascontiguousarray(
            b_[g * OG : (g + 1) * OG].astype(np.float32).reshape(2, 128).T
        )

    wq_g = [wslice(wq_w, g) for g in range(GROUPS)]
    wk_g = [wslice(wk_w, g) for g in range(GROUPS)]
    wv_g = [wslice(wv_w, g) for g in range(GROUPS)]
    qb_g = [bslice(wq_b, g) for g in range(GROUPS)]
    kb_g = [bslice(wk_b, g) for g in range(GROUPS)]
    wd_g = []
    for g in range(GROUPS):
        ds = dense_w[:, g * OG : (g + 1) * OG].T.astype(NPBF16)  # [OG, D]
        wd_g.append(np.ascontiguousarray(ds.reshape(2, 128, D_MODEL).transpose(1, 0, 2)))

    in_maps = []
    for c in range(N_CORES):
        b, g = c // GROUPS, c % GROUPS
        in_maps.append(
            {
                "xq": xq_b[b],
                "xk": xk_b[b],
                "xv": xv_b[b],
                "wq": wq_g[g],
                "wk": wk_g[g],
                "wv": wv_g[g],
                "wd": wd_g[g],
                "qb": qb_g[g],
                "kb": kb_g[g],
                "mk": mk_arr,
            }
        )

    kw = dict(trace=True, **TRACE_KW) if TRACE else {}
    res = run_bass_kernel_spmd(nc, in_maps, core_ids=list(range(N_CORES)), **kw)
    LAST_RESULT = res

    corr = dense_w @ wv_b + dense_b  # v-bias pushed through dense, + dense bias
    out = np.empty((B, S, D_MODEL), dtype=np.float32)
    for b in range(B):
        acc = np.zeros((S, D_MODEL), dtype=np.float32)
        for g in range(GROUPS):
            acc += res.results[b * GROUPS + g]["outp"]
        out[b] = acc + corr
    return out


# revision 27
# speedup vs baseline: 1.0088x; 1.0088x over previous
"""Multi-head attention (B=2, S=2048, D=1024, H=16) on 8 TRN2 NeuronCores.

Sharding: core c handles batch c//4 and head-group c%4 (4 heads each).
Host pre-transposes inputs/weights to d-major bf16; each core computes its
4 heads' projections, causal attention, and a partial (row-parallel) dense
output [S, D] which the host sums across the 4 cores of each batch.

Attention math: scores are computed transposed ([k, q] layout, q on the
free dim) so no on-chip transposes are ever needed.  Masking is applied as
a multiplicative factor F = exp(-1e9*m/8) on the exp'd scores (exact for
0/1 masks, correct in general); fully-masked 128x512 tiles are skipped at
emit time based on the actual mask contents.  Softmax row sums come for
free from a ones-column appended to the V tiles in the AV matmul; the
reciprocal is broadcast across partitions via a small DRAM bounce.
"""

import numpy as np
import ml_dtypes
from contextlib import ExitStack

import concourse.bass as bass
import concourse.tile as tile
from concourse import bacc, mybir
from concourse.bass_utils import run_bass_kernel_spmd

BF16 = mybir.dt.bfloat16
F32 = mybir.dt.float32
NPBF16 = ml_dtypes.bfloat16

D_MODEL = 1024
NH = 16
DEPTH = 64
B = 2
S = 2048
N_CORES = 8
GROUPS = 4              # head-groups (tensor parallel dimension)
HPG = NH // GROUPS      # 4 heads per core
OG = HPG * DEPTH        # 256 projection output cols per core
QC = 512                # q chunk (matmul free dim)
NQC = S // QC           # 4
KT = 128                # k tile (psum partition dim)
NKT = S // KT           # 16
DK = D_MODEL // 128     # 8 contraction tiles of 128
SC = 512                # projection s chunk
NSC = S // SC           # 4
EGRP = 3                # k-tiles per exp group (psum group tile)

TRACE = False
TRACE_KW = {}
LAST_RESULT = None
DEBUG = False
_CACHE = {}


def _chunk(lst, n):
    return [lst[i : i + n] for i in range(0, len(lst), n)]


def _build(ktiles, mults, n_uniq):
    """Emit the bass program. ktiles[j] = list of computed k-tiles for
    q-chunk j; mults[j][t] = mask-factor tile id (or absent)."""
    nc = bacc.Bacc(
        "TRN2", target_bir_lowering=False, debug=False, num_devices=N_CORES
    )
    xq = nc.dram_tensor("xq", [128, DK, S], BF16, kind="ExternalInput").ap()
    xk = nc.dram_tensor("xk", [128, DK, S], BF16, kind="ExternalInput").ap()
    xv = nc.dram_tensor("xv", [128, DK, S], BF16, kind="ExternalInput").ap()
    wq = nc.dram_tensor("wq", [128, DK, OG], BF16, kind="ExternalInput").ap()
    wk = nc.dram_tensor("wk", [128, DK, OG], BF16, kind="ExternalInput").ap()
    wv = nc.dram_tensor("wv", [128, DK, OG], BF16, kind="ExternalInput").ap()
    wd = nc.dram_tensor("wd", [128, 2, D_MODEL], BF16, kind="ExternalInput").ap()
    qb = nc.dram_tensor("qb", [128, 2], F32, kind="ExternalInput").ap()
    kb = nc.dram_tensor("kb", [128, 2], F32, kind="ExternalInput").ap()
    mk = nc.dram_tensor("mk", [128, n_uniq, QC], BF16, kind="ExternalInput").ap()
    outp = nc.dram_tensor("outp", [S, D_MODEL], F32, kind="ExternalOutput").ap()

    Ident = mybir.ActivationFunctionType.Identity
    Exp = mybir.ActivationFunctionType.Exp

    with tile.TileContext(nc) as tc, ExitStack() as ctx:
        singles = ctx.enter_context(tc.tile_pool(name="singles", bufs=1))
        exps = ctx.enter_context(tc.tile_pool(name="exps", bufs=3))
        small = ctx.enter_context(tc.tile_pool(name="small", bufs=2))
        bcastp = ctx.enter_context(tc.tile_pool(name="bcastp", bufs=3))
        dram = ctx.enter_context(tc.tile_pool(name="dram", bufs=2, space="DRAM"))

        wq_sb = singles.tile([128, DK, OG], BF16)
        nc.sync.dma_start(wq_sb[:], wq)
        wk_sb = singles.tile([128, DK, OG], BF16)
        nc.sync.dma_start(wk_sb[:], wk)
        wv_sb = singles.tile([128, DK, OG], BF16)
        nc.sync.dma_start(wv_sb[:], wv)
        wd_sb = singles.tile([128, 2, D_MODEL], BF16)
        nc.sync.dma_start(wd_sb[:], wd)
        mk_sb = singles.tile([128, n_uniq, QC], BF16)
        nc.sync.dma_start(mk_sb[:], mk)
        qb_sb = singles.tile([128, 2], F32)
        nc.sync.dma_start(qb_sb[:], qb)
        kb_sb = singles.tile([128, 2], F32)
        nc.sync.dma_start(kb_sb[:], kb)

        # o = co*128 + p layouts
        qt = singles.tile([128, 2, S], BF16)
        kt_ = singles.tile([128, 2, S], BF16)
        # [p=k%128, ktile, head, 64 d cols + ones col]
        vh1 = singles.tile([128, NKT, HPG, 65], BF16)
        avf = singles.tile([128, 2, S], F32)    # unnormalized av^T
        avb = singles.tile([128, 2, S], BF16)   # normalized av^T

        nc.vector.memset(vh1[:, :, :, 64:65], 1.0)

        # ---- projections (own psum + input-staging scope) ----
        # inputs streamed in contiguous S-halves (fat DMA descriptors)
        SH = S // 2
        with tc.tile_pool(name="xin", bufs=3) as xin, tc.tile_pool(
            name="pp", bufs=3, space="PSUM"
        ) as pp:
            # q/k: out[o, s] = w[:, o].T @ x[:, s]
            for x_ap, w_sb, b_sb, dst in (
                (xq, wq_sb, qb_sb, qt),
                (xk, wk_sb, kb_sb, kt_),
            ):
                for half in range(2):
                    x_sb = xin.tile([128, DK, SH], BF16, tag="xin")
                    nc.sync.dma_start(
                        x_sb[:], x_ap[:, :, half * SH : (half + 1) * SH]
                    )
                    for sch in range(SH // SC):
                        sc = half * (SH // SC) + sch
                        for oc in range(2):
                            ps = pp.tile([128, SC], F32, tag="pp")
                            for dk in range(DK):
                                nc.tensor.matmul(
                                    ps[:],
                                    lhsT=w_sb[:, dk, oc * 128 : (oc + 1) * 128],
                                    rhs=x_sb[:, dk, sch * SC : (sch + 1) * SC],
                                    start=(dk == 0),
                                    stop=(dk == DK - 1),
                                )
                            nc.vector.tensor_scalar(
                                out=dst[:, oc, sc * SC : (sc + 1) * SC],
                                in0=ps[:],
                                scalar1=b_sb[:, oc : oc + 1],
                                scalar2=None,
                                op0=mybir.AluOpType.add,
                            )

            # v: out[s, o] = x[:, s].T @ w[:, o]
            for half in range(2):
                xv_sb = xin.tile([128, DK, SH], BF16, tag="xin")
                nc.sync.dma_start(xv_sb[:], xv[:, :, half * SH : (half + 1) * SH])
                for sth in range(SH // KT):
                    st = half * (SH // KT) + sth
                    ps = pp.tile([128, SC], F32, tag="pp")
                    for dk in range(DK):
                        nc.tensor.matmul(
                            ps[:, :OG],
                            lhsT=xv_sb[:, dk, sth * KT : (sth + 1) * KT],
                            rhs=wv_sb[:, dk, :],
                            start=(dk == 0),
                            stop=(dk == DK - 1),
                        )
                    nc.vector.tensor_copy(
                        out=vh1[:, st, :, 0:64],
                        in_=ps[:, :OG].rearrange("p (h d) -> p h d", d=DEPTH),
                    )

        # ---- attention, per local head ----
        attn_ctx = ExitStack()
        psc = attn_ctx.enter_context(
            tc.tile_pool(name="psc", bufs=2, space="PSUM")
        )
        pav = attn_ctx.enter_context(
            tc.tile_pool(name="pav", bufs=2, space="PSUM")
        )
        dbg_rec = (
            nc.dram_tensor("dbg_rec", [HPG, NQC, QC], F32, kind="ExternalOutput").ap()
            if DEBUG
            else None
        )
        dbg_den = (
            nc.dram_tensor("dbg_den", [HPG, NQC, QC], F32, kind="ExternalOutput").ap()
            if DEBUG
            else None
        )
        dbg_bc = (
            nc.dram_tensor(
                "dbg_bc", [HPG, NQC, 128, QC], F32, kind="ExternalOutput"
            ).ap()
            if DEBUG
            else None
        )
        for h in range(HPG):
            odd = h % 2
            pb = odd * 64
            ch = h // 2
            den_stage = small.tile([128, NQC, QC], F32, tag="denst")
            for j in range(NQC):
                tiles = ktiles[j]
                first, last = tiles[0], tiles[-1]
                ps_av = pav.tile([128, QC], F32, tag="pav")
                for grp in _chunk(tiles, EGRP):
                    ps_g = psc.tile([128, EGRP, QC], F32, tag="psc")
                    for r, t in enumerate(grp):
                        nc.tensor.matmul(
                            ps_g[:, r, :],
                            lhsT=kt_[pb : pb + 64, ch, t * KT : (t + 1) * KT],
                            rhs=qt[pb : pb + 64, ch, j * QC : (j + 1) * QC],
                            start=True,
                            stop=True,
                        )
                    ex = exps.tile([128, EGRP, QC], BF16, tag="exps")
                    nc.scalar.activation(
                        out=ex[:, : len(grp), :],
                        in_=ps_g[:, : len(grp), :],
                        func=Exp,
                        scale=0.125,
                    )
                    for r, t in enumerate(grp):
                        uid = mults[j].get(t)
                        if uid is not None:
                            nc.gpsimd.tensor_mul(
                                ex[:, r, :], ex[:, r, :], mk_sb[:, uid, :]
                            )
                    for r, t in enumerate(grp):
                        nc.tensor.matmul(
                            ps_av[0:65, :],
                            lhsT=vh1[:, t, h, :],
                            rhs=ex[:, r, :],
                            start=(t == first),
                            stop=(t == last),
                        )
                if odd:
                    # engines can't shift partitions; bounce via SBUF + DMA
                    tmp = bcastp.tile([64, QC], F32, tag="avtmp")
                    nc.vector.tensor_copy(out=tmp[:], in_=ps_av[0:64, :])
                    nc.sync.dma_start(
                        avf[64:128, ch, j * QC : (j + 1) * QC], tmp[:]
                    )
                else:
                    nc.vector.tensor_copy(
                        out=avf[0:64, ch, j * QC : (j + 1) * QC],
                        in_=ps_av[0:64, :],
                    )
                nc.vector.tensor_copy(
                    out=den_stage[64:65, j, :], in_=ps_av[64:65, :]
                )
            # custom DVE recip op requires base partition 0 -> DMA-shift rows
            den0 = small.tile([NQC, QC], F32, tag="den0")
            nc.sync.dma_start(den0[:], den_stage[64:65, :, :])
            rec0 = small.tile([NQC, QC], F32, tag="rec0")
            nc.vector.reciprocal_approx_fast(rec0[:], den0[:])
            rec_t = dram.tile([NQC, QC], F32, tag="rec")
            nc.sync.dma_start(rec_t[:], rec0[:])
            if DEBUG:
                nc.sync.dma_start(dbg_rec[h : h + 1, :, :], rec0[:])
                nc.sync.dma_start(dbg_den[h : h + 1, :, :], den_stage[64:65, :, :])
            for j in range(NQC):
                bc = bcastp.tile([128, QC], F32, tag="bc")
                nc.sync.dma_start(
                    bc[pb : pb + 64, :],
                    rec_t[j : j + 1, :].to_broadcast([64, QC]),
                )
                if DEBUG:
                    nc.sync.dma_start(dbg_bc[h, j, pb : pb + 64, :], bc[pb : pb + 64, :])
                nc.vector.tensor_mul(
                    avb[pb : pb + 64, ch, j * QC : (j + 1) * QC],
                    avf[pb : pb + 64, ch, j * QC : (j + 1) * QC],
                    bc[pb : pb + 64, :],
                )

        attn_ctx.close()

        # ---- dense (row-parallel partial): out[s, :] = av^T.T @ wd ----
        with tc.tile_pool(name="pd", bufs=3, space="PSUM") as pd, tc.tile_pool(
            name="ost", bufs=3
        ) as ost:
            for st in range(NKT):
                ot = ost.tile([128, D_MODEL], F32, tag="ostage")
                for oc in range(2):
                    ps = pd.tile([128, SC], F32, tag="pd")
                    for co in range(2):
                        nc.tensor.matmul(
                            ps[:],
                            lhsT=avb[:, co, st * 128 : (st + 1) * 128],
                            rhs=wd_sb[:, co, oc * 512 : (oc + 1) * 512],
                            start=(co == 0),
                            stop=(co == 1),
                        )
                    nc.vector.tensor_copy(
                        out=ot[:, oc * 512 : (oc + 1) * 512], in_=ps[:]
                    )
                nc.sync.dma_start(outp[st * 128 : (st + 1) * 128, :], ot[:])

        if DEBUG:
            for name, t in (
                ("dbg_qt", qt),
                ("dbg_kt", kt_),
                ("dbg_vh1", vh1),
                ("dbg_avf", avf),
                ("dbg_avb", avb),
            ):
                dt_ = nc.dram_tensor(
                    name, list(t.shape), t.dtype, kind="ExternalOutput"
                ).ap()
                nc.sync.dma_start(dt_, t[:])

    nc.compile()
    return nc


def _classify_mask(mask):
    """Classify 128(k) x 512(q) score tiles from the actual mask contents.

    Returns (ktiles, mults, mk_arr):
      ktiles[j]: k-tile indices to compute for q-chunk j
      mults[j]: {t: unique factor tile id}
      mk_arr: [128, NU, 512] bf16 multiplicative factors exp(-1e9*m/8)
    """
    m2 = np.asarray(mask, dtype=np.float32).reshape(S, S)
    F = np.exp(m2 * np.float32(-1.25e8))  # exp(-1e9*m/8); 0/1 masks -> 0/1
    if (F.max(axis=1) == 0.0).any():
        raise RuntimeError("mask has fully-masked rows; unsupported")
    blocks = F.reshape(NKT, 128, NKT, 128)  # [qi, qr, t, kr]
    kept = (blocks == 1.0).all(axis=(1, 3))  # [qi, t]
    skip = (blocks == 0.0).all(axis=(1, 3))

    ktiles = []
    mults = []
    uniq = {}
    mk_tiles = []
    for j in range(NQC):
        qis = range(j * (QC // 128), (j + 1) * (QC // 128))
        tl = []
        mu = {}
        for t in range(NKT):
            if all(skip[qi, t] for qi in qis):
                continue
            tl.append(t)
            if all(kept[qi, t] for qi in qis):
                continue
            fb = np.ascontiguousarray(
                F[j * QC : (j + 1) * QC, t * KT : (t + 1) * KT].T
            ).astype(NPBF16)
            key = fb.tobytes()
            if key not in uniq:
                uniq[key] = len(mk_tiles)
                mk_tiles.append(fb)
            mu[t] = uniq[key]
        if not tl:
            raise RuntimeError("q-chunk with no kept k-tiles; unsupported")
        ktiles.append(tl)
        mults.append(mu)
    if not mk_tiles:
        mk_tiles.append(np.ones((128, QC), dtype=NPBF16))
    mk_arr = np.ascontiguousarray(np.stack(mk_tiles, axis=0).transpose(1, 0, 2))
    return ktiles, mults, mk_arr


def _xt_prep(x):
    """[S, D] f32 -> [128, DK, S] bf16, d-major (d = do*128 + di)."""
    xt = x.T.astype(NPBF16)  # [D, S]
    return np.ascontiguousarray(xt.reshape(DK, 128, S).transpose(1, 0, 2))


def kernel(v, k, q, mask, wq_w, wq_b, wk_w, wk_b, wv_w, wv_b, dense_w, dense_b):
    global LAST_RESULT
    v = np.asarray(v, dtype=np.float32)
    k = np.asarray(k, dtype=np.float32)
    q = np.asarray(q, dtype=np.float32)
    mask = np.asarray(mask, dtype=np.float32)
    wq_w = np.asarray(wq_w, dtype=np.float32)
    wk_w = np.asarray(wk_w, dtype=np.float32)
    wv_w = np.asarray(wv_w, dtype=np.float32)
    dense_w = np.asarray(dense_w, dtype=np.float32)
    wq_b = np.asarray(wq_b, dtype=np.float32)
    wk_b = np.asarray(wk_b, dtype=np.float32)
    wv_b = np.asarray(wv_b, dtype=np.float32)
    dense_b = np.asarray(dense_b, dtype=np.float32)

    ktiles, mults, mk_arr = _classify_mask(mask)
    key = (
        tuple(tuple(t) for t in ktiles),
        tuple(tuple(sorted(m.items())) for m in mults),
        mk_arr.shape[1],
    )
    if key not in _CACHE:
        _CACHE[key] = _build(ktiles, mults, mk_arr.shape[1])
    nc = _CACHE[key]

    # per-batch inputs (shared by the 4 cores of each batch)
    xq_b = [_xt_prep(q[b]) for b in range(B)]
    xk_b = [_xt_prep(k[b]) for b in range(B)]
    xv_b = [_xt_prep(v[b]) for b in range(B)]

    # per-group weights
    def wslice(w, g):
        ws = w[g * OG : (g + 1) * OG, :].T.astype(NPBF16)  # [D, OG]
        return np.ascontiguousarray(ws.reshape(DK, 128, OG).transpose(1, 0, 2))

    def bslice(b_, g):
        return np.ascontiguousarray(
            b_[g * OG : (g + 1) * OG].astype(np.float32).reshape(2, 128).T
        )

    wq_g = [wslice(wq_w, g) for g in range(GROUPS)]
    wk_g = [wslice(wk_w, g) for g in range(GROUPS)]
    wv_g = [wslice(wv_w, g) for g in range(GROUPS)]
    qb_g = [bslice(wq_b, g) for g in range(GROUPS)]
    kb_g = [bslice(wk_b, g) for g in range(GROUPS)]
    wd_g = []
    for g in range(GROUPS):
        ds = dense_w[:, g * OG : (g + 1) * OG].T.astype(NPBF16)  # [OG, D]
        wd_g.append(np.ascontiguousarray(ds.reshape(2, 128, D_MODEL).transpose(1, 0, 2)))

    in_maps = []
    for c in range(N_CORES):
        b, g = c // GROUPS, c % GROUPS
        in_maps.append(
            {
                "xq": xq_b[b],
                "xk": xk_b[b],
                "xv": xv_b[b],
                "wq": wq_g[g],
                "wk": wk_g[g],
                "wv": wv_g[g],
                "wd": wd_g[g],
                "qb": qb_g[g],
                "kb": kb_g[g],
                "mk": mk_arr,
            }
        )

    kw = dict(trace=True, **TRACE_KW) if TRACE else {}
    res = run_bass_kernel_spmd(nc, in_maps, core_ids=list(range(N_CORES)), **kw)
    LAST_RESULT = res

    corr = dense_w @ wv_b + dense_b  # v-bias pushed through dense, + dense bias
    out = np.empty((B, S, D_MODEL), dtype=np.float32)
    for b in range(B):
        acc = np.zeros((S, D_MODEL), dtype=np.float32)
        for g in range(GROUPS):
            acc += res.results[b * GROUPS + g]["outp"]
        out[b] = acc + corr
    return out


# revision 32
# speedup vs baseline: 1.1232x; 1.1134x over previous
"""Multi-head attention (B=2, S=2048, D=1024, H=16) on 8 TRN2 NeuronCores.

Sharding: core c handles batch c//4 and head-group c%4 (4 heads each).
Host pre-transposes inputs/weights to d-major bf16; each core computes its
4 heads' projections, causal attention, and a partial (row-parallel) dense
output [S, D] which the host sums across the 4 cores of each batch.

Attention math: scores are computed transposed ([k, q] layout, q on the
free dim) so no on-chip transposes are ever needed.  Masking is applied as
a multiplicative factor F = exp(-1e9*m/8) on the exp'd scores (exact for
0/1 masks, correct in general); fully-masked 128x512 tiles are skipped at
emit time based on the actual mask contents.  Softmax row sums come for
free from a ones-column appended to the V tiles in the AV matmul; the
reciprocal is broadcast across partitions via a small DRAM bounce.
"""

import numpy as np
import ml_dtypes
from contextlib import ExitStack

import concourse.bass as bass
import concourse.tile as tile
from concourse import bacc, mybir
from concourse.bass_utils import run_bass_kernel_spmd

BF16 = mybir.dt.bfloat16
F32 = mybir.dt.float32
NPBF16 = ml_dtypes.bfloat16

D_MODEL = 1024
NH = 16
DEPTH = 64
B = 2
S = 2048
N_CORES = 8
GROUPS = 4              # head-groups (tensor parallel dimension)
HPG = NH // GROUPS      # 4 heads per core
OG = HPG * DEPTH        # 256 projection output cols per core
QC = 512                # q chunk (matmul free dim)
NQC = S // QC           # 4
KT = 128                # k tile (psum partition dim)
NKT = S // KT           # 16
DK = D_MODEL // 128     # 8 contraction tiles of 128
SC = 512                # projection s chunk
NSC = S // SC           # 4
EGRP = 3                # k-tiles per exp group (psum group tile)

TRACE = False
TRACE_KW = {}
LAST_RESULT = None
DEBUG = False
_CACHE = {}


def _chunk(lst, n):
    return [lst[i : i + n] for i in range(0, len(lst), n)]


def _build(ktiles, mults, n_uniq):
    """Emit the bass program. ktiles[j] = list of computed k-tiles for
    q-chunk j; mults[j][t] = mask-factor tile id (or absent)."""
    nc = bacc.Bacc(
        "TRN2", target_bir_lowering=False, debug=False, num_devices=N_CORES
    )
    # inputs pre-split into contiguous S-halves for fat DMA descriptors
    xq = nc.dram_tensor("xq", [2, 128, DK, S // 2], BF16, kind="ExternalInput").ap()
    xk = nc.dram_tensor("xk", [2, 128, DK, S // 2], BF16, kind="ExternalInput").ap()
    xv = nc.dram_tensor("xv", [2, 128, DK, S // 2], BF16, kind="ExternalInput").ap()
    wq = nc.dram_tensor("wq", [128, DK, OG], BF16, kind="ExternalInput").ap()
    wk = nc.dram_tensor("wk", [128, DK, OG], BF16, kind="ExternalInput").ap()
    wv = nc.dram_tensor("wv", [128, DK, OG], BF16, kind="ExternalInput").ap()
    wd = nc.dram_tensor("wd", [128, 2, D_MODEL], BF16, kind="ExternalInput").ap()
    qb = nc.dram_tensor("qb", [128, 2], F32, kind="ExternalInput").ap()
    kb = nc.dram_tensor("kb", [128, 2], F32, kind="ExternalInput").ap()
    mk = nc.dram_tensor("mk", [128, n_uniq, QC], BF16, kind="ExternalInput").ap()
    outp = nc.dram_tensor("outp", [S, D_MODEL], F32, kind="ExternalOutput").ap()

    Ident = mybir.ActivationFunctionType.Identity
    Exp = mybir.ActivationFunctionType.Exp

    with tile.TileContext(nc) as tc, ExitStack() as ctx:
        singles = ctx.enter_context(tc.tile_pool(name="singles", bufs=1))
        exps = ctx.enter_context(tc.tile_pool(name="exps", bufs=3))
        small = ctx.enter_context(tc.tile_pool(name="small", bufs=2))
        bcastp = ctx.enter_context(tc.tile_pool(name="bcastp", bufs=3))
        dram = ctx.enter_context(tc.tile_pool(name="dram", bufs=2, space="DRAM"))

        wq_sb = singles.tile([128, DK, OG], BF16)
        nc.sync.dma_start(wq_sb[:], wq)
        wk_sb = singles.tile([128, DK, OG], BF16)
        nc.sync.dma_start(wk_sb[:], wk)
        wv_sb = singles.tile([128, DK, OG], BF16)
        nc.sync.dma_start(wv_sb[:], wv)
        wd_sb = singles.tile([128, 2, D_MODEL], BF16)
        nc.sync.dma_start(wd_sb[:], wd)
        mk_sb = singles.tile([128, n_uniq, QC], BF16)
        nc.sync.dma_start(mk_sb[:], mk)
        qb_sb = singles.tile([128, 2], F32)
        nc.sync.dma_start(qb_sb[:], qb)
        kb_sb = singles.tile([128, 2], F32)
        nc.sync.dma_start(kb_sb[:], kb)

        # o = co*128 + p layouts
        qt = singles.tile([128, 2, S], BF16)
        kt_ = singles.tile([128, 2, S], BF16)
        # [p=k%128, ktile, head, 64 d cols + ones col]
        vh1 = singles.tile([128, NKT, HPG, 65], BF16)
        avf = singles.tile([128, 2, S], F32)    # unnormalized av^T
        avb = singles.tile([128, 2, S], BF16)   # normalized av^T

        nc.vector.memset(vh1[:, :, :, 64:65], 1.0)

        # ---- projections (own psum + input-staging scope) ----
        # inputs streamed in contiguous S-halves (fat DMA descriptors)
        SH = S // 2
        with tc.tile_pool(name="xin", bufs=3) as xin, tc.tile_pool(
            name="pp", bufs=3, space="PSUM"
        ) as pp:
            # q/k: out[o, s] = w[:, o].T @ x[:, s]
            for x_ap, w_sb, b_sb, dst in (
                (xq, wq_sb, qb_sb, qt),
                (xk, wk_sb, kb_sb, kt_),
            ):
                for half in range(2):
                    x_sb = xin.tile([128, DK, SH], BF16, tag="xin")
                    nc.sync.dma_start(x_sb[:], x_ap[half])
                    for sch in range(SH // SC):
                        sc = half * (SH // SC) + sch
                        for oc in range(2):
                            ps = pp.tile([128, SC], F32, tag="pp")
                            for dk in range(DK):
                                nc.tensor.matmul(
                                    ps[:],
                                    lhsT=w_sb[:, dk, oc * 128 : (oc + 1) * 128],
                                    rhs=x_sb[:, dk, sch * SC : (sch + 1) * SC],
                                    start=(dk == 0),
                                    stop=(dk == DK - 1),
                                )
                            nc.vector.tensor_scalar(
                                out=dst[:, oc, sc * SC : (sc + 1) * SC],
                                in0=ps[:],
                                scalar1=b_sb[:, oc : oc + 1],
                                scalar2=None,
                                op0=mybir.AluOpType.add,
                            )

            # v: out[s, o] = x[:, s].T @ w[:, o]
            for half in range(2):
                xv_sb = xin.tile([128, DK, SH], BF16, tag="xin")
                nc.sync.dma_start(xv_sb[:], xv[half])
                for sth in range(SH // KT):
                    st = half * (SH // KT) + sth
                    ps = pp.tile([128, SC], F32, tag="pp")
                    for dk in range(DK):
                        nc.tensor.matmul(
                            ps[:, :OG],
                            lhsT=xv_sb[:, dk, sth * KT : (sth + 1) * KT],
                            rhs=wv_sb[:, dk, :],
                            start=(dk == 0),
                            stop=(dk == DK - 1),
                        )
                    nc.vector.tensor_copy(
                        out=vh1[:, st, :, 0:64],
                        in_=ps[:, :OG].rearrange("p (h d) -> p h d", d=DEPTH),
                    )

        # ---- attention, per local head ----
        attn_ctx = ExitStack()
        psc = attn_ctx.enter_context(
            tc.tile_pool(name="psc", bufs=2, space="PSUM")
        )
        pav = attn_ctx.enter_context(
            tc.tile_pool(name="pav", bufs=2, space="PSUM")
        )
        dbg_rec = (
            nc.dram_tensor("dbg_rec", [HPG, NQC, QC], F32, kind="ExternalOutput").ap()
            if DEBUG
            else None
        )
        dbg_den = (
            nc.dram_tensor("dbg_den", [HPG, NQC, QC], F32, kind="ExternalOutput").ap()
            if DEBUG
            else None
        )
        dbg_bc = (
            nc.dram_tensor(
                "dbg_bc", [HPG, NQC, 128, QC], F32, kind="ExternalOutput"
            ).ap()
            if DEBUG
            else None
        )
        for h in range(HPG):
            odd = h % 2
            pb = odd * 64
            ch = h // 2
            den_stage = small.tile([128, NQC, QC], F32, tag="denst")
            for j in range(NQC):
                tiles = ktiles[j]
                first, last = tiles[0], tiles[-1]
                ps_av = pav.tile([128, QC], F32, tag="pav")
                for grp in _chunk(tiles, EGRP):
                    ps_g = psc.tile([128, EGRP, QC], F32, tag="psc")
                    for r, t in enumerate(grp):
                        nc.tensor.matmul(
                            ps_g[:, r, :],
                            lhsT=kt_[pb : pb + 64, ch, t * KT : (t + 1) * KT],
                            rhs=qt[pb : pb + 64, ch, j * QC : (j + 1) * QC],
                            start=True,
                            stop=True,
                        )
                    ex = exps.tile([128, EGRP, QC], BF16, tag="exps")
                    nc.scalar.activation(
                        out=ex[:, : len(grp), :],
                        in_=ps_g[:, : len(grp), :],
                        func=Exp,
                        scale=0.125,
                    )
                    for r, t in enumerate(grp):
                        uid = mults[j].get(t)
                        if uid is not None:
                            eng = nc.vector if t % 2 == 0 else nc.gpsimd
                            eng.tensor_mul(
                                ex[:, r, :], ex[:, r, :], mk_sb[:, uid, :]
                            )
                    for r, t in enumerate(grp):
                        nc.tensor.matmul(
                            ps_av[0:65, :],
                            lhsT=vh1[:, t, h, :],
                            rhs=ex[:, r, :],
                            start=(t == first),
                            stop=(t == last),
                        )
                if odd:
                    # engines can't shift partitions; bounce via SBUF + DMA
                    tmp = bcastp.tile([64, QC], F32, tag="avtmp")
                    nc.vector.tensor_copy(out=tmp[:], in_=ps_av[0:64, :])
                    nc.sync.dma_start(
                        avf[64:128, ch, j * QC : (j + 1) * QC], tmp[:]
                    )
                else:
                    nc.vector.tensor_copy(
                        out=avf[0:64, ch, j * QC : (j + 1) * QC],
                        in_=ps_av[0:64, :],
                    )
                nc.vector.tensor_copy(
                    out=den_stage[64:65, j, :], in_=ps_av[64:65, :]
                )
            # custom DVE recip op requires base partition 0 -> DMA-shift rows
            den0 = small.tile([NQC, QC], F32, tag="den0")
            nc.sync.dma_start(den0[:], den_stage[64:65, :, :])
            rec0 = small.tile([NQC, QC], F32, tag="rec0")
            nc.vector.reciprocal_approx_fast(rec0[:], den0[:])
            rec_t = dram.tile([NQC, QC], F32, tag="rec")
            nc.sync.dma_start(rec_t[:], rec0[:])
            if DEBUG:
                nc.sync.dma_start(dbg_rec[h : h + 1, :, :], rec0[:])
                nc.sync.dma_start(dbg_den[h : h + 1, :, :], den_stage[64:65, :, :])
            for j in range(NQC):
                bc = bcastp.tile([128, QC], F32, tag="bc")
                nc.sync.dma_start(
                    bc[pb : pb + 64, :],
                    rec_t[j : j + 1, :].to_broadcast([64, QC]),
                )
                if DEBUG:
                    nc.sync.dma_start(dbg_bc[h, j, pb : pb + 64, :], bc[pb : pb + 64, :])
                nc.vector.tensor_mul(
                    avb[pb : pb + 64, ch, j * QC : (j + 1) * QC],
                    avf[pb : pb + 64, ch, j * QC : (j + 1) * QC],
                    bc[pb : pb + 64, :],
                )

        attn_ctx.close()

        # ---- dense (row-parallel partial): out[s, :] = av^T.T @ wd ----
        with tc.tile_pool(name="pd", bufs=3, space="PSUM") as pd, tc.tile_pool(
            name="ost", bufs=3
        ) as ost:
            for st in range(NKT):
                ot = ost.tile([128, D_MODEL], F32, tag="ostage")
                for oc in range(2):
                    ps = pd.tile([128, SC], F32, tag="pd")
                    for co in range(2):
                        nc.tensor.matmul(
                            ps[:],
                            lhsT=avb[:, co, st * 128 : (st + 1) * 128],
                            rhs=wd_sb[:, co, oc * 512 : (oc + 1) * 512],
                            start=(co == 0),
                            stop=(co == 1),
                        )
                    nc.vector.tensor_copy(
                        out=ot[:, oc * 512 : (oc + 1) * 512], in_=ps[:]
                    )
                nc.sync.dma_start(outp[st * 128 : (st + 1) * 128, :], ot[:])

        if DEBUG:
            for name, t in (
                ("dbg_qt", qt),
                ("dbg_kt", kt_),
                ("dbg_vh1", vh1),
                ("dbg_avf", avf),
                ("dbg_avb", avb),
            ):
                dt_ = nc.dram_tensor(
                    name, list(t.shape), t.dtype, kind="ExternalOutput"
                ).ap()
                nc.sync.dma_start(dt_, t[:])

    nc.compile()
    return nc


def _classify_mask(mask):
    """Classify 128(k) x 512(q) score tiles from the actual mask contents.

    Returns (ktiles, mults, mk_arr):
      ktiles[j]: k-tile indices to compute for q-chunk j
      mults[j]: {t: unique factor tile id}
      mk_arr: [128, NU, 512] bf16 multiplicative factors exp(-1e9*m/8)
    """
    m2 = np.asarray(mask, dtype=np.float32).reshape(S, S)
    F = np.exp(m2 * np.float32(-1.25e8))  # exp(-1e9*m/8); 0/1 masks -> 0/1
    if (F.max(axis=1) == 0.0).any():
        raise RuntimeError("mask has fully-masked rows; unsupported")
    blocks = F.reshape(NKT, 128, NKT, 128)  # [qi, qr, t, kr]
    kept = (blocks == 1.0).all(axis=(1, 3))  # [qi, t]
    skip = (blocks == 0.0).all(axis=(1, 3))

    ktiles = []
    mults = []
    uniq = {}
    mk_tiles = []
    for j in range(NQC):
        qis = range(j * (QC // 128), (j + 1) * (QC // 128))
        tl = []
        mu = {}
        for t in range(NKT):
            if all(skip[qi, t] for qi in qis):
                continue
            tl.append(t)
            if all(kept[qi, t] for qi in qis):
                continue
            fb = np.ascontiguousarray(
                F[j * QC : (j + 1) * QC, t * KT : (t + 1) * KT].T
            ).astype(NPBF16)
            key = fb.tobytes()
            if key not in uniq:
                uniq[key] = len(mk_tiles)
                mk_tiles.append(fb)
            mu[t] = uniq[key]
        if not tl:
            raise RuntimeError("q-chunk with no kept k-tiles; unsupported")
        ktiles.append(tl)
        mults.append(mu)
    if not mk_tiles:
        mk_tiles.append(np.ones((128, QC), dtype=NPBF16))
    mk_arr = np.ascontiguousarray(np.stack(mk_tiles, axis=0).transpose(1, 0, 2))
    return ktiles, mults, mk_arr


def _xt_prep(x):
    """[S, D] f32 -> [2, 128, DK, S/2] bf16, d-major, contiguous S-halves."""
    xt = x.T.astype(NPBF16)  # [D, S]
    a = xt.reshape(DK, 128, 2, S // 2).transpose(2, 1, 0, 3)
    return np.ascontiguousarray(a)


def kernel(v, k, q, mask, wq_w, wq_b, wk_w, wk_b, wv_w, wv_b, dense_w, dense_b):
    global LAST_RESULT
    v = np.asarray(v, dtype=np.float32)
    k = np.asarray(k, dtype=np.float32)
    q = np.asarray(q, dtype=np.float32)
    mask = np.asarray(mask, dtype=np.float32)
    wq_w = np.asarray(wq_w, dtype=np.float32)
    wk_w = np.asarray(wk_w, dtype=np.float32)
    wv_w = np.asarray(wv_w, dtype=np.float32)
    dense_w = np.asarray(dense_w, dtype=np.float32)
    wq_b = np.asarray(wq_b, dtype=np.float32)
    wk_b = np.asarray(wk_b, dtype=np.float32)
    wv_b = np.asarray(wv_b, dtype=np.float32)
    dense_b = np.asarray(dense_b, dtype=np.float32)

    ktiles, mults, mk_arr = _classify_mask(mask)
    key = (
        tuple(tuple(t) for t in ktiles),
        tuple(tuple(sorted(m.items())) for m in mults),
        mk_arr.shape[1],
    )
    if key not in _CACHE:
        _CACHE[key] = _build(ktiles, mults, mk_arr.shape[1])
    nc = _CACHE[key]

    # per-batch inputs (shared by the 4 cores of each batch)
    xq_b = [_xt_prep(q[b]) for b in range(B)]
    xk_b = [_xt_prep(k[b]) for b in range(B)]
    xv_b = [_xt_prep(v[b]) for b in range(B)]

    # per-group weights
    def wslice(w, g):
        ws = w[g * OG : (g + 1) * OG, :].T.astype(NPBF16)  # [D, OG]
        return np.ascontiguousarray(ws.reshape(DK, 128, OG).transpose(1, 0, 2))

    def bslice(b_, g):
        return np.ascontiguousarray(
            b_[g * OG : (g + 1) * OG].astype(np.float32).reshape(2, 128).T
        )

    wq_g = [wslice(wq_w, g) for g in range(GROUPS)]
    wk_g = [wslice(wk_w, g) for g in range(GROUPS)]
    wv_g = [wslice(wv_w, g) for g in range(GROUPS)]
    qb_g = [bslice(wq_b, g) for g in range(GROUPS)]
    kb_g = [bslice(wk_b, g) for g in range(GROUPS)]
    wd_g = []
    for g in range(GROUPS):
        ds = dense_w[:, g * OG : (g + 1) * OG].T.astype(NPBF16)  # [OG, D]
        wd_g.append(np.ascontiguousarray(ds.reshape(2, 128, D_MODEL).transpose(1, 0, 2)))

    in_maps = []
    for c in range(N_CORES):
        b, g = c // GROUPS, c % GROUPS
        in_maps.append(
            {
                "xq": xq_b[b],
                "xk": xk_b[b],
                "xv": xv_b[b],
                "wq": wq_g[g],
                "wk": wk_g[g],
                "wv": wv_g[g],
                "wd": wd_g[g],
                "qb": qb_g[g],
                "kb": kb_g[g],
                "mk": mk_arr,
            }
        )

    kw = dict(trace=True, **TRACE_KW) if TRACE else {}
    res = run_bass_kernel_spmd(nc, in_maps, core_ids=list(range(N_CORES)), **kw)
    LAST_RESULT = res

    corr = dense_w @ wv_b + dense_b  # v-bias pushed through dense, + dense bias
    out = np.empty((B, S, D_MODEL), dtype=np.float32)
    for b in range(B):
        acc = np.zeros((S, D_MODEL), dtype=np.float32)
        for g in range(GROUPS):
            acc += res.results[b * GROUPS + g]["outp"]
        out[b] = acc + corr
    return out


# revision 40
# speedup vs baseline: 1.1246x; 1.0013x over previous
"""Multi-head attention (B=2, S=2048, D=1024, H=16) on 8 TRN2 NeuronCores.

Sharding: core c handles batch c//4 and head-group c%4 (4 heads each).
Host pre-transposes inputs/weights to d-major bf16; each core computes its
4 heads' projections, causal attention, and a partial (row-parallel) dense
output [S, D] which the host sums across the 4 cores of each batch.

Attention math: scores are computed transposed ([k, q] layout, q on the
free dim) so no on-chip transposes are ever needed.  Masking is applied as
a multiplicative factor F = exp(-1e9*m/8) on the exp'd scores (exact for
0/1 masks, correct in general); fully-masked 128x512 tiles are skipped at
emit time based on the actual mask contents.  Softmax row sums come for
free from a ones-column appended to the V tiles in the AV matmul; the
reciprocal is broadcast across partitions via a small DRAM bounce.
"""

import numpy as np
import ml_dtypes
from contextlib import ExitStack

import concourse.bass as bass
import concourse.tile as tile
from concourse import bacc, mybir
from concourse.bass_utils import run_bass_kernel_spmd

BF16 = mybir.dt.bfloat16
F32 = mybir.dt.float32
NPBF16 = ml_dtypes.bfloat16

D_MODEL = 1024
NH = 16
DEPTH = 64
B = 2
S = 2048
N_CORES = 8
GROUPS = 4              # head-groups (tensor parallel dimension)
HPG = NH // GROUPS      # 4 heads per core
OG = HPG * DEPTH        # 256 projection output cols per core
QC = 512                # q chunk (matmul free dim)
NQC = S // QC           # 4
KT = 128                # k tile (psum partition dim)
NKT = S // KT           # 16
DK = D_MODEL // 128     # 8 contraction tiles of 128
SC = 512                # projection s chunk
NSC = S // SC           # 4
EGRP = 3                # k-tiles per exp group (psum group tile)

TRACE = False
TRACE_KW = {}
LAST_RESULT = None
DEBUG = False
_CACHE = {}


def _chunk(lst, n):
    return [lst[i : i + n] for i in range(0, len(lst), n)]


def _build(ktiles, mults, n_uniq):
    """Emit the bass program. ktiles[j] = list of computed k-tiles for
    q-chunk j; mults[j][t] = mask-factor tile id (or absent)."""
    nc = bacc.Bacc(
        "TRN2", target_bir_lowering=False, debug=False, num_devices=N_CORES
    )
    # inputs pre-split into contiguous S-quarters for fat DMA descriptors
    xq = nc.dram_tensor("xq", [NSC, 128, DK, SC], BF16, kind="ExternalInput").ap()
    xk = nc.dram_tensor("xk", [NSC, 128, DK, SC], BF16, kind="ExternalInput").ap()
    xv = nc.dram_tensor("xv", [NSC, 128, DK, SC], BF16, kind="ExternalInput").ap()
    wq = nc.dram_tensor("wq", [128, DK, OG], BF16, kind="ExternalInput").ap()
    wk = nc.dram_tensor("wk", [128, DK, OG], BF16, kind="ExternalInput").ap()
    wv = nc.dram_tensor("wv", [128, DK, OG], BF16, kind="ExternalInput").ap()
    wd = nc.dram_tensor("wd", [128, 2, D_MODEL], BF16, kind="ExternalInput").ap()
    qb = nc.dram_tensor("qb", [128, 2], F32, kind="ExternalInput").ap()
    kb = nc.dram_tensor("kb", [128, 2], F32, kind="ExternalInput").ap()
    mk = nc.dram_tensor("mk", [128, n_uniq, QC], BF16, kind="ExternalInput").ap()
    outp = nc.dram_tensor("outp", [S, D_MODEL], F32, kind="ExternalOutput").ap()

    Ident = mybir.ActivationFunctionType.Identity
    Exp = mybir.ActivationFunctionType.Exp

    with tile.TileContext(nc) as tc, ExitStack() as ctx:
        singles = ctx.enter_context(tc.tile_pool(name="singles", bufs=1))
        exps = ctx.enter_context(tc.tile_pool(name="exps", bufs=3))
        small = ctx.enter_context(tc.tile_pool(name="small", bufs=2))
        bcastp = ctx.enter_context(tc.tile_pool(name="bcastp", bufs=3))
        dram = ctx.enter_context(tc.tile_pool(name="dram", bufs=2, space="DRAM"))

        wq_sb = singles.tile([128, DK, OG], BF16)
        nc.sync.dma_start(wq_sb[:], wq)
        wk_sb = singles.tile([128, DK, OG], BF16)
        nc.sync.dma_start(wk_sb[:], wk)
        wv_sb = singles.tile([128, DK, OG], BF16)
        nc.sync.dma_start(wv_sb[:], wv)
        mk_sb = singles.tile([128, n_uniq, QC], BF16)
        nc.sync.dma_start(mk_sb[:], mk)
        qb_sb = singles.tile([128, 2], F32)
        nc.sync.dma_start(qb_sb[:], qb)
        kb_sb = singles.tile([128, 2], F32)
        nc.sync.dma_start(kb_sb[:], kb)
        wd_sb = singles.tile([128, 2, D_MODEL], BF16)  # loaded late, before dense

        # o = co*128 + p layouts
        qt = singles.tile([128, 2, S], BF16)
        kt_ = singles.tile([128, 2, S], BF16)
        # [p=k%128, ktile, head, 64 d cols + ones col]
        vh1 = singles.tile([128, NKT, HPG, 65], BF16)
        avf = singles.tile([128, 2, S], F32)    # unnormalized av^T
        avb = singles.tile([128, 2, S], BF16)   # normalized av^T

        nc.vector.memset(vh1[:, :, :, 64:65], 1.0)

        # ---- projections (own psum + input-staging scope) ----
        # inputs streamed in contiguous S-quarters (fat DMA descriptors);
        # xin bufs throttle prefetch so loads land in consumption order
        with tc.tile_pool(name="xin", bufs=3) as xin, tc.tile_pool(
            name="pp", bufs=3, space="PSUM"
        ) as pp:
            # q/k: out[o, s] = w[:, o].T @ x[:, s]
            for x_ap, w_sb, b_sb, dst in (
                (xq, wq_sb, qb_sb, qt),
                (xk, wk_sb, kb_sb, kt_),
            ):
                for sc in range(NSC):
                    x_sb = xin.tile([128, DK, SC], BF16, tag="xin")
                    nc.sync.dma_start(x_sb[:], x_ap[sc])
                    for oc in range(2):
                        ps = pp.tile([128, SC], F32, tag="pp")
                        for dk in range(DK):
                            nc.tensor.matmul(
                                ps[:],
                                lhsT=w_sb[:, dk, oc * 128 : (oc + 1) * 128],
                                rhs=x_sb[:, dk, :],
                                start=(dk == 0),
                                stop=(dk == DK - 1),
                            )
                        nc.vector.tensor_scalar(
                            out=dst[:, oc, sc * SC : (sc + 1) * SC],
                            in0=ps[:],
                            scalar1=b_sb[:, oc : oc + 1],
                            scalar2=None,
                            op0=mybir.AluOpType.add,
                        )

            # v: out[s, o] = x[:, s].T @ w[:, o]
            for sc in range(NSC):
                xv_sb = xin.tile([128, DK, SC], BF16, tag="xin")
                nc.sync.dma_start(xv_sb[:], xv[sc])
                for sth in range(SC // KT):
                    st = sc * (SC // KT) + sth
                    ps = pp.tile([128, SC], F32, tag="pp")
                    for dk in range(DK):
                        nc.tensor.matmul(
                            ps[:, :OG],
                            lhsT=xv_sb[:, dk, sth * KT : (sth + 1) * KT],
                            rhs=wv_sb[:, dk, :],
                            start=(dk == 0),
                            stop=(dk == DK - 1),
                        )
                    nc.vector.tensor_copy(
                        out=vh1[:, st, :, 0:64],
                        in_=ps[:, :OG].rearrange("p (h d) -> p h d", d=DEPTH),
                    )

        # ---- attention, per local head ----
        nc.sync.dma_start(wd_sb[:], wd)  # prefetch for the dense phase
        attn_ctx = ExitStack()
        psc = attn_ctx.enter_context(
            tc.tile_pool(name="psc", bufs=2, space="PSUM")
        )
        pav = attn_ctx.enter_context(
            tc.tile_pool(name="pav", bufs=2, space="PSUM")
        )
        dbg_rec = (
            nc.dram_tensor("dbg_rec", [HPG, NQC, QC], F32, kind="ExternalOutput").ap()
            if DEBUG
            else None
        )
        dbg_den = (
            nc.dram_tensor("dbg_den", [HPG, NQC, QC], F32, kind="ExternalOutput").ap()
            if DEBUG
            else None
        )
        dbg_bc = (
            nc.dram_tensor(
                "dbg_bc", [HPG, NQC, 128, QC], F32, kind="ExternalOutput"
            ).ap()
            if DEBUG
            else None
        )
        for h in range(HPG):
            odd = h % 2
            pb = odd * 64
            ch = h // 2
            den_stage = small.tile([128, NQC, QC], F32, tag="denst")
            for j in range(NQC):
                tiles = ktiles[j]
                first, last = tiles[0], tiles[-1]
                ps_av = pav.tile([128, QC], F32, tag="pav")
                for grp in _chunk(tiles, EGRP):
                    ps_g = psc.tile([128, EGRP, QC], F32, tag="psc")
                    for r, t in enumerate(grp):
                        nc.tensor.matmul(
                            ps_g[:, r, :],
                            lhsT=kt_[pb : pb + 64, ch, t * KT : (t + 1) * KT],
                            rhs=qt[pb : pb + 64, ch, j * QC : (j + 1) * QC],
                            start=True,
                            stop=True,
                        )
                    ex = exps.tile([128, EGRP, QC], BF16, tag="exps")
                    nc.scalar.activation(
                        out=ex[:, : len(grp), :],
                        in_=ps_g[:, : len(grp), :],
                        func=Exp,
                        scale=0.125,
                    )
                    rhs_t = {}
                    exm = None
                    for r, t in enumerate(grp):
                        uid = mults[j].get(t)
                        if uid is None:
                            rhs_t[t] = ex[:, r, :]
                        else:
                            # out != in keeps DVE in fast 2x mode
                            if exm is None:
                                exm = exps.tile([128, EGRP, QC], BF16, tag="exm")
                            eng = nc.vector if t % 2 == 0 else nc.gpsimd
                            eng.tensor_mul(
                                exm[:, r, :], ex[:, r, :], mk_sb[:, uid, :]
                            )
                            rhs_t[t] = exm[:, r, :]
                    for r, t in enumerate(grp):
                        nc.tensor.matmul(
                            ps_av[0:65, :],
                            lhsT=vh1[:, t, h, :],
                            rhs=rhs_t[t],
                            start=(t == first),
                            stop=(t == last),
                        )
                if odd:
                    # engines can't shift partitions; bounce via SBUF + DMA
                    tmp = bcastp.tile([64, QC], F32, tag="avtmp")
                    nc.vector.tensor_copy(out=tmp[:], in_=ps_av[0:64, :])
                    nc.sync.dma_start(
                        avf[64:128, ch, j * QC : (j + 1) * QC], tmp[:]
                    )
                else:
                    nc.vector.tensor_copy(
                        out=avf[0:64, ch, j * QC : (j + 1) * QC],
                        in_=ps_av[0:64, :],
                    )
                nc.vector.tensor_copy(
                    out=den_stage[64:65, j, :], in_=ps_av[64:65, :]
                )
            # custom DVE recip op requires base partition 0 -> DMA-shift rows
            den0 = small.tile([NQC, QC], F32, tag="den0")
            nc.sync.dma_start(den0[:], den_stage[64:65, :, :])
            rec0 = small.tile([NQC, QC], F32, tag="rec0")
            nc.vector.reciprocal_approx_fast(rec0[:], den0[:])
            rec_t = dram.tile([NQC, QC], F32, tag="rec")
            nc.sync.dma_start(rec_t[:], rec0[:])
            if DEBUG:
                nc.sync.dma_start(dbg_rec[h : h + 1, :, :], rec0[:])
                nc.sync.dma_start(dbg_den[h : h + 1, :, :], den_stage[64:65, :, :])
            for j in range(NQC):
                bc = bcastp.tile([128, QC], F32, tag="bc")
                nc.sync.dma_start(
                    bc[pb : pb + 64, :],
                    rec_t[j : j + 1, :].to_broadcast([64, QC]),
                )
                if DEBUG:
                    nc.sync.dma_start(dbg_bc[h, j, pb : pb + 64, :], bc[pb : pb + 64, :])
                nc.vector.tensor_mul(
                    avb[pb : pb + 64, ch, j * QC : (j + 1) * QC],
                    avf[pb : pb + 64, ch, j * QC : (j + 1) * QC],
                    bc[pb : pb + 64, :],
                )

        attn_ctx.close()

        # ---- dense (row-parallel partial): out[s, :] = av^T.T @ wd ----
        with tc.tile_pool(name="pd", bufs=3, space="PSUM") as pd, tc.tile_pool(
            name="ost", bufs=3
        ) as ost:
            for st in range(NKT):
                ot = ost.tile([128, D_MODEL], F32, tag="ostage")
                for oc in range(2):
                    ps = pd.tile([128, SC], F32, tag="pd")
                    for co in range(2):
                        nc.tensor.matmul(
                            ps[:],
                            lhsT=avb[:, co, st * 128 : (st + 1) * 128],
                            rhs=wd_sb[:, co, oc * 512 : (oc + 1) * 512],
                            start=(co == 0),
                            stop=(co == 1),
                        )
                    if oc == 0:
                        nc.vector.tensor_copy(
                            out=ot[:, oc * 512 : (oc + 1) * 512], in_=ps[:]
                        )
                    else:
                        nc.scalar.copy(
                            out=ot[:, oc * 512 : (oc + 1) * 512], in_=ps[:]
                        )
                nc.sync.dma_start(outp[st * 128 : (st + 1) * 128, :], ot[:])

        if DEBUG:
            for name, t in (
                ("dbg_qt", qt),
                ("dbg_kt", kt_),
                ("dbg_vh1", vh1),
                ("dbg_avf", avf),
                ("dbg_avb", avb),
            ):
                dt_ = nc.dram_tensor(
                    name, list(t.shape), t.dtype, kind="ExternalOutput"
                ).ap()
                nc.sync.dma_start(dt_, t[:])

    nc.compile()
    return nc


def _classify_mask(mask):
    """Classify 128(k) x 512(q) score tiles from the actual mask contents.

    Returns (ktiles, mults, mk_arr):
      ktiles[j]: k-tile indices to compute for q-chunk j
      mults[j]: {t: unique factor tile id}
      mk_arr: [128, NU, 512] bf16 multiplicative factors exp(-1e9*m/8)
    """
    m2 = np.asarray(mask, dtype=np.float32).reshape(S, S)
    F = np.exp(m2 * np.float32(-1.25e8))  # exp(-1e9*m/8); 0/1 masks -> 0/1
    if (F.max(axis=1) == 0.0).any():
        raise RuntimeError("mask has fully-masked rows; unsupported")
    blocks = F.reshape(NKT, 128, NKT, 128)  # [qi, qr, t, kr]
    kept = (blocks == 1.0).all(axis=(1, 3))  # [qi, t]
    skip = (blocks == 0.0).all(axis=(1, 3))

    ktiles = []
    mults = []
    uniq = {}
    mk_tiles = []
    for j in range(NQC):
        qis = range(j * (QC // 128), (j + 1) * (QC // 128))
        tl = []
        mu = {}
        for t in range(NKT):
            if all(skip[qi, t] for qi in qis):
                continue
            tl.append(t)
            if all(kept[qi, t] for qi in qis):
                continue
            fb = np.ascontiguousarray(
                F[j * QC : (j + 1) * QC, t * KT : (t + 1) * KT].T
            ).astype(NPBF16)
            key = fb.tobytes()
            if key not in uniq:
                uniq[key] = len(mk_tiles)
                mk_tiles.append(fb)
            mu[t] = uniq[key]
        if not tl:
            raise RuntimeError("q-chunk with no kept k-tiles; unsupported")
        ktiles.append(tl)
        mults.append(mu)
    if not mk_tiles:
        mk_tiles.append(np.ones((128, QC), dtype=NPBF16))
    mk_arr = np.ascontiguousarray(np.stack(mk_tiles, axis=0).transpose(1, 0, 2))
    return ktiles, mults, mk_arr


def _xt_prep(x):
    """[S, D] f32 -> [NSC, 128, DK, SC] bf16, d-major, contiguous S-quarters."""
    xt = x.T.astype(NPBF16)  # [D, S]
    a = xt.reshape(DK, 128, NSC, SC).transpose(2, 1, 0, 3)
    return np.ascontiguousarray(a)


def kernel(v, k, q, mask, wq_w, wq_b, wk_w, wk_b, wv_w, wv_b, dense_w, dense_b):
    global LAST_RESULT
    v = np.asarray(v, dtype=np.float32)
    k = np.asarray(k, dtype=np.float32)
    q = np.asarray(q, dtype=np.float32)
    mask = np.asarray(mask, dtype=np.float32)
    wq_w = np.asarray(wq_w, dtype=np.float32)
    wk_w = np.asarray(wk_w, dtype=np.float32)
    wv_w = np.asarray(wv_w, dtype=np.float32)
    dense_w = np.asarray(dense_w, dtype=np.float32)
    wq_b = np.asarray(wq_b, dtype=np.float32)
    wk_b = np.asarray(wk_b, dtype=np.float32)
    wv_b = np.asarray(wv_b, dtype=np.float32)
    dense_b = np.asarray(dense_b, dtype=np.float32)

    ktiles, mults, mk_arr = _classify_mask(mask)
    key = (
        tuple(tuple(t) for t in ktiles),
        tuple(tuple(sorted(m.items())) for m in mults),
        mk_arr.shape[1],
    )
    if key not in _CACHE:
        _CACHE[key] = _build(ktiles, mults, mk_arr.shape[1])
    nc = _CACHE[key]

    # per-batch inputs (shared by the 4 cores of each batch)
    xq_b = [_xt_prep(q[b]) for b in range(B)]
    xk_b = [_xt_prep(k[b]) for b in range(B)]
    xv_b = [_xt_prep(v[b]) for b in range(B)]

    # per-group weights
    def wslice(w, g):
        ws = w[g * OG : (g + 1) * OG, :].T.astype(NPBF16)  # [D, OG]
        return np.ascontiguousarray(ws.reshape(DK, 128, OG).transpose(1, 0, 2))

    def bslice(b_, g):
        return np.ascontiguousarray(
            b_[g * OG : (g + 1) * OG].astype(np.float32).reshape(2, 128).T
        )

    wq_g = [wslice(wq_w, g) for g in range(GROUPS)]
    wk_g = [wslice(wk_w, g) for g in range(GROUPS)]
    wv_g = [wslice(wv_w, g) for g in range(GROUPS)]
    qb_g = [bslice(wq_b, g) for g in range(GROUPS)]
    kb_g = [bslice(wk_b, g) for g in range(GROUPS)]
    wd_g = []
    for g in range(GROUPS):
        ds = dense_w[:, g * OG : (g + 1) * OG].T.astype(NPBF16)  # [OG, D]
        wd_g.append(np.ascontiguousarray(ds.reshape(2, 128, D_MODEL).transpose(1, 0, 2)))

    in_maps = []
    for c in range(N_CORES):
        b, g = c // GROUPS, c % GROUPS
        in_maps.append(
            {
                "xq": xq_b[b],
                "xk": xk_b[b],
                "xv": xv_b[b],
                "wq": wq_g[g],
                "wk": wk_g[g],
                "wv": wv_g[g],
                "wd": wd_g[g],
                "qb": qb_g[g],
                "kb": kb_g[g],
                "mk": mk_arr,
            }
        )

    kw = dict(trace=True, **TRACE_KW) if TRACE else {}
    res = run_bass_kernel_spmd(nc, in_maps, core_ids=list(range(N_CORES)), **kw)
    LAST_RESULT = res

    corr = dense_w @ wv_b + dense_b  # v-bias pushed through dense, + dense bias
    out = np.empty((B, S, D_MODEL), dtype=np.float32)
    for b in range(B):
        acc = np.zeros((S, D_MODEL), dtype=np.float32)
        for g in range(GROUPS):
            acc += res.results[b * GROUPS + g]["outp"]
        out[b] = acc + corr
    return out


# revision 46
# speedup vs baseline: 1.1398x; 1.0135x over previous
"""Multi-head attention (B=2, S=2048, D=1024, H=16) on 8 TRN2 NeuronCores.

Sharding: core c handles batch c//4 and head-group c%4 (4 heads each).
Host pre-transposes inputs/weights to d-major bf16; each core computes its
4 heads' projections, causal attention, and a partial (row-parallel) dense
output [S, D] which the host sums across the 4 cores of each batch.

Attention math: scores are computed transposed ([k, q] layout, q on the
free dim) so no on-chip transposes are ever needed.  Masking is applied as
a multiplicative factor F = exp(-1e9*m/8) on the exp'd scores (exact for
0/1 masks, correct in general); fully-masked 128x512 tiles are skipped at
emit time based on the actual mask contents.  Softmax row sums come for
free from a ones-column appended to the V tiles in the AV matmul; the
reciprocal is broadcast across partitions via a small DRAM bounce.
"""

import numpy as np
import ml_dtypes
from contextlib import ExitStack

import concourse.bass as bass
import concourse.tile as tile
from concourse import bacc, mybir
from concourse.bass_utils import run_bass_kernel_spmd

BF16 = mybir.dt.bfloat16
F32 = mybir.dt.float32
NPBF16 = ml_dtypes.bfloat16

D_MODEL = 1024
NH = 16
DEPTH = 64
B = 2
S = 2048
N_CORES = 8
GROUPS = 4              # head-groups (tensor parallel dimension)
HPG = NH // GROUPS      # 4 heads per core
OG = HPG * DEPTH        # 256 projection output cols per core
QC = 512                # q chunk (matmul free dim)
NQC = S // QC           # 4
KT = 128                # k tile (psum partition dim)
NKT = S // KT           # 16
DK = D_MODEL // 128     # 8 contraction tiles of 128
SC = 512                # projection s chunk
NSC = S // SC           # 4
EGRP = 3                # k-tiles per exp group (psum group tile)

TRACE = False
TRACE_KW = {}
LAST_RESULT = None
DEBUG = False
_CACHE = {}


def _chunk(lst, n):
    return [lst[i : i + n] for i in range(0, len(lst), n)]


def _build(ktiles, n_uniq):
    """Emit the bass program. ktiles[j] = [(t, lo, tri), ...] computed
    k-tiles for q-chunk j (see _classify_mask)."""
    nc = bacc.Bacc(
        "TRN2", target_bir_lowering=False, debug=False, num_devices=N_CORES
    )
    # inputs pre-split into contiguous S-quarters for fat DMA descriptors
    xq = nc.dram_tensor("xq", [NSC, 128, DK, SC], BF16, kind="ExternalInput").ap()
    xk = nc.dram_tensor("xk", [NSC, 128, DK, SC], BF16, kind="ExternalInput").ap()
    xv = nc.dram_tensor("xv", [NSC, 128, DK, SC], BF16, kind="ExternalInput").ap()
    wq = nc.dram_tensor("wq", [128, DK, OG], BF16, kind="ExternalInput").ap()
    wk = nc.dram_tensor("wk", [128, DK, OG], BF16, kind="ExternalInput").ap()
    wv = nc.dram_tensor("wv", [128, DK, OG], BF16, kind="ExternalInput").ap()
    wd = nc.dram_tensor("wd", [128, 2, D_MODEL], BF16, kind="ExternalInput").ap()
    qb = nc.dram_tensor("qb", [128, 2], F32, kind="ExternalInput").ap()
    kb = nc.dram_tensor("kb", [128, 2], F32, kind="ExternalInput").ap()
    mk = nc.dram_tensor("mk", [128, n_uniq, KT], BF16, kind="ExternalInput").ap()
    outp = nc.dram_tensor("outp", [S, D_MODEL], F32, kind="ExternalOutput").ap()

    Ident = mybir.ActivationFunctionType.Identity
    Exp = mybir.ActivationFunctionType.Exp

    with tile.TileContext(nc) as tc, ExitStack() as ctx:
        singles = ctx.enter_context(tc.tile_pool(name="singles", bufs=1))
        exps = ctx.enter_context(tc.tile_pool(name="exps", bufs=3))
        small = ctx.enter_context(tc.tile_pool(name="small", bufs=2))
        bcastp = ctx.enter_context(tc.tile_pool(name="bcastp", bufs=3))
        dram = ctx.enter_context(tc.tile_pool(name="dram", bufs=2, space="DRAM"))

        wq_sb = singles.tile([128, DK, OG], BF16)
        nc.sync.dma_start(wq_sb[:], wq)
        wk_sb = singles.tile([128, DK, OG], BF16)
        nc.sync.dma_start(wk_sb[:], wk)
        wv_sb = singles.tile([128, DK, OG], BF16)
        nc.sync.dma_start(wv_sb[:], wv)
        mk_sb = singles.tile([128, n_uniq, KT], BF16)
        nc.sync.dma_start(mk_sb[:], mk)
        qb_sb = singles.tile([128, 2], F32)
        nc.sync.dma_start(qb_sb[:], qb)
        kb_sb = singles.tile([128, 2], F32)
        nc.sync.dma_start(kb_sb[:], kb)
        wd_sb = singles.tile([128, 2, D_MODEL], BF16)  # loaded late, before dense

        # o = co*128 + p layouts
        qt = singles.tile([128, 2, S], BF16)
        kt_ = singles.tile([128, 2, S], BF16)
        # [p=k%128, ktile, head, 64 d cols + ones col]
        vh1 = singles.tile([128, NKT, HPG, 65], BF16)
        avf = singles.tile([128, 2, S], F32)    # unnormalized av^T
        avb = singles.tile([128, 2, S], BF16)   # normalized av^T

        nc.vector.memset(vh1[:, :, :, 64:65], 1.0)

        # ---- projections (own psum + input-staging scope) ----
        # inputs streamed in contiguous S-quarters (fat DMA descriptors);
        # xin bufs throttle prefetch so loads land in consumption order
        with tc.tile_pool(name="xin", bufs=3) as xin, tc.tile_pool(
            name="pp", bufs=3, space="PSUM"
        ) as pp:
            # q/k: out[o, s] = w[:, o].T @ x[:, s]
            for x_ap, w_sb, b_sb, dst in (
                (xq, wq_sb, qb_sb, qt),
                (xk, wk_sb, kb_sb, kt_),
            ):
                for sc in range(NSC):
                    x_sb = xin.tile([128, DK, SC], BF16, tag="xin")
                    nc.sync.dma_start(x_sb[:], x_ap[sc])
                    for oc in range(2):
                        ps = pp.tile([128, SC], F32, tag="pp")
                        for dk in range(DK):
                            nc.tensor.matmul(
                                ps[:],
                                lhsT=w_sb[:, dk, oc * 128 : (oc + 1) * 128],
                                rhs=x_sb[:, dk, :],
                                start=(dk == 0),
                                stop=(dk == DK - 1),
                            )
                        nc.vector.tensor_scalar(
                            out=dst[:, oc, sc * SC : (sc + 1) * SC],
                            in0=ps[:],
                            scalar1=b_sb[:, oc : oc + 1],
                            scalar2=None,
                            op0=mybir.AluOpType.add,
                        )

            # v: out[s, o] = x[:, s].T @ w[:, o]
            for sc in range(NSC):
                xv_sb = xin.tile([128, DK, SC], BF16, tag="xin")
                nc.sync.dma_start(xv_sb[:], xv[sc])
                for sth in range(SC // KT):
                    st = sc * (SC // KT) + sth
                    ps = pp.tile([128, SC], F32, tag="pp")
                    for dk in range(DK):
                        nc.tensor.matmul(
                            ps[:, :OG],
                            lhsT=xv_sb[:, dk, sth * KT : (sth + 1) * KT],
                            rhs=wv_sb[:, dk, :],
                            start=(dk == 0),
                            stop=(dk == DK - 1),
                        )
                    nc.vector.tensor_copy(
                        out=vh1[:, st, :, 0:64],
                        in_=ps[:, :OG].rearrange("p (h d) -> p h d", d=DEPTH),
                    )

        # ---- attention, per local head ----
        nc.sync.dma_start(wd_sb[:], wd)  # prefetch for the dense phase
        attn_ctx = ExitStack()
        psc = attn_ctx.enter_context(
            tc.tile_pool(name="psc", bufs=2, space="PSUM")
        )
        pav = attn_ctx.enter_context(
            tc.tile_pool(name="pav", bufs=2, space="PSUM")
        )
        dbg_rec = (
            nc.dram_tensor("dbg_rec", [HPG, NQC, QC], F32, kind="ExternalOutput").ap()
            if DEBUG
            else None
        )
        dbg_den = (
            nc.dram_tensor("dbg_den", [HPG, NQC, QC], F32, kind="ExternalOutput").ap()
            if DEBUG
            else None
        )
        dbg_bc = (
            nc.dram_tensor(
                "dbg_bc", [HPG, NQC, 128, QC], F32, kind="ExternalOutput"
            ).ap()
            if DEBUG
            else None
        )
        for h in range(HPG):
            odd = h % 2
            pb = odd * 64
            ch = h // 2
            den_stage = small.tile([128, NQC, QC], F32, tag="denst")
            for j in range(NQC):
                tiles = ktiles[j]
                first, last = tiles[0][0], tiles[-1][0]
                ps_av = pav.tile([128, QC], F32, tag="pav")
                for grp in _chunk(tiles, EGRP):
                    ps_g = psc.tile([128, EGRP, QC], F32, tag="psc")
                    for r, (t, lo, tri) in enumerate(grp):
                        # cols [0, lo*128) are fully masked: never computed,
                        # never read by the av matmul below
                        nc.tensor.matmul(
                            ps_g[:, r, lo * 128 :],
                            lhsT=kt_[pb : pb + 64, ch, t * KT : (t + 1) * KT],
                            rhs=qt[
                                pb : pb + 64,
                                ch,
                                j * QC + lo * 128 : (j + 1) * QC,
                            ],
                            start=True,
                            stop=True,
                        )
                    ex = exps.tile([128, EGRP, QC], BF16, tag="exps")
                    nc.scalar.activation(
                        out=ex[:, : len(grp), :],
                        in_=ps_g[:, : len(grp), :],
                        func=Exp,
                        scale=0.125,
                    )
                    for r, (t, lo, tri) in enumerate(grp):
                        for i, uid in tri:
                            nc.vector.tensor_mul(
                                ex[:, r, i * 128 : (i + 1) * 128],
                                ex[:, r, i * 128 : (i + 1) * 128],
                                mk_sb[:, uid, :],
                            )
                    for r, (t, lo, tri) in enumerate(grp):
                        nc.tensor.matmul(
                            ps_av[0:65, lo * 128 :],
                            lhsT=vh1[:, t, h, :],
                            rhs=ex[:, r, lo * 128 :],
                            start=(t == first),
                            stop=(t == last),
                        )
                if odd:
                    # engines can't shift partitions; bounce via SBUF + DMA
                    tmp = bcastp.tile([64, QC], F32, tag="avtmp")
                    nc.vector.tensor_copy(out=tmp[:], in_=ps_av[0:64, :])
                    nc.sync.dma_start(
                        avf[64:128, ch, j * QC : (j + 1) * QC], tmp[:]
                    )
                else:
                    nc.vector.tensor_copy(
                        out=avf[0:64, ch, j * QC : (j + 1) * QC],
                        in_=ps_av[0:64, :],
                    )
                nc.vector.tensor_copy(
                    out=den_stage[64:65, j, :], in_=ps_av[64:65, :]
                )
            # custom DVE recip op requires base partition 0 -> DMA-shift rows
            den0 = small.tile([NQC, QC], F32, tag="den0")
            nc.sync.dma_start(den0[:], den_stage[64:65, :, :])
            rec0 = small.tile([NQC, QC], F32, tag="rec0")
            nc.vector.reciprocal_approx_fast(rec0[:], den0[:])
            rec_t = dram.tile([NQC, QC], F32, tag="rec")
            nc.sync.dma_start(rec_t[:], rec0[:])
            if DEBUG:
                nc.sync.dma_start(dbg_rec[h : h + 1, :, :], rec0[:])
                nc.sync.dma_start(dbg_den[h : h + 1, :, :], den_stage[64:65, :, :])
            for j in range(NQC):
                bc = bcastp.tile([128, QC], F32, tag="bc")
                nc.sync.dma_start(
                    bc[pb : pb + 64, :],
                    rec_t[j : j + 1, :].to_broadcast([64, QC]),
                )
                if DEBUG:
                    nc.sync.dma_start(dbg_bc[h, j, pb : pb + 64, :], bc[pb : pb + 64, :])
                nc.vector.tensor_mul(
                    avb[pb : pb + 64, ch, j * QC : (j + 1) * QC],
                    avf[pb : pb + 64, ch, j * QC : (j + 1) * QC],
                    bc[pb : pb + 64, :],
                )

        attn_ctx.close()

        # ---- dense (row-parallel partial): out[s, :] = av^T.T @ wd ----
        with tc.tile_pool(name="pd", bufs=3, space="PSUM") as pd, tc.tile_pool(
            name="ost", bufs=3
        ) as ost:
            for st in range(NKT):
                ot = ost.tile([128, D_MODEL], F32, tag="ostage")
                for oc in range(2):
                    ps = pd.tile([128, SC], F32, tag="pd")
                    for co in range(2):
                        nc.tensor.matmul(
                            ps[:],
                            lhsT=avb[:, co, st * 128 : (st + 1) * 128],
                            rhs=wd_sb[:, co, oc * 512 : (oc + 1) * 512],
                            start=(co == 0),
                            stop=(co == 1),
                        )
                    if oc == 0:
                        nc.vector.tensor_copy(
                            out=ot[:, oc * 512 : (oc + 1) * 512], in_=ps[:]
                        )
                    else:
                        nc.scalar.copy(
                            out=ot[:, oc * 512 : (oc + 1) * 512], in_=ps[:]
                        )
                nc.sync.dma_start(outp[st * 128 : (st + 1) * 128, :], ot[:])

        if DEBUG:
            for name, t in (
                ("dbg_qt", qt),
                ("dbg_kt", kt_),
                ("dbg_vh1", vh1),
                ("dbg_avf", avf),
                ("dbg_avb", avb),
            ):
                dt_ = nc.dram_tensor(
                    name, list(t.shape), t.dtype, kind="ExternalOutput"
                ).ap()
                nc.sync.dma_start(dt_, t[:])

    nc.compile()
    return nc


def _classify_mask(mask):
    """Classify 128(k) x 128(q) score blocks from the actual mask contents.

    Returns (ktiles, mk_arr):
      ktiles[j]: list of (t, lo, tri) per computed k-tile for q-chunk j:
        lo: first kept 128-col block within the 512-wide q-chunk (cols
            [0, lo*128) are fully masked and simply never computed/read)
        tri: [(col_block, uid), ...] 128-col blocks needing a factor mult
      mk_arr: [128, NU, 128] bf16 multiplicative factors exp(-1e9*m/8)
    """
    m2 = np.asarray(mask, dtype=np.float32).reshape(S, S)
    F = np.exp(m2 * np.float32(-1.25e8))  # exp(-1e9*m/8); 0/1 masks -> 0/1
    if (F.max(axis=1) == 0.0).any():
        raise RuntimeError("mask has fully-masked rows; unsupported")
    blocks = F.reshape(NKT, 128, NKT, 128)  # [qi, qr, t, kr]
    kept = (blocks == 1.0).all(axis=(1, 3))  # [qi, t]
    skip = (blocks == 0.0).all(axis=(1, 3))

    NB = QC // 128  # 128-col blocks per q-chunk
    ktiles = []
    uniq = {}
    mk_tiles = []

    def factor_uid(qi, t):
        fb = np.ascontiguousarray(
            F[qi * 128 : (qi + 1) * 128, t * KT : (t + 1) * KT].T
        ).astype(NPBF16)
        key = fb.tobytes()
        if key not in uniq:
            uniq[key] = len(mk_tiles)
            mk_tiles.append(fb)
        return uniq[key]

    for j in range(NQC):
        qis = list(range(j * NB, (j + 1) * NB))
        tl = []
        for t in range(NKT):
            stats = [
                "k" if kept[qi, t] else ("s" if skip[qi, t] else "m")
                for qi in qis
            ]
            if all(s == "s" for s in stats):
                continue
            lo = next(i for i, s in enumerate(stats) if s != "s")
            tri = []
            for i in range(lo, NB):
                if stats[i] == "k":
                    continue
                # mixed OR interior skip (multiply by its factor / zeros)
                tri.append((i, factor_uid(qis[i], t)))
            tl.append((t, lo, tri))
        if not tl:
            raise RuntimeError("q-chunk with no kept k-tiles; unsupported")
        # the first computed tile must span the full chunk (av 'start' MM)
        if tl[0][1] != 0:
            t0, _, tri0 = tl[0]
            tri0 = [(i, u) for i, u in tri0]
            have = {i for i, _ in tri0}
            for i in range(tl[0][1]):
                if i not in have:
                    tri0.append((i, factor_uid(qis[i], t0)))
            tl[0] = (t0, 0, sorted(tri0))
        ktiles.append(tl)
    if not mk_tiles:
        mk_tiles.append(np.ones((128, KT), dtype=NPBF16))
    mk_arr = np.ascontiguousarray(np.stack(mk_tiles, axis=0).transpose(1, 0, 2))
    return ktiles, mk_arr


def _xt_prep(x):
    """[S, D] f32 -> [NSC, 128, DK, SC] bf16, d-major, contiguous S-quarters."""
    xt = x.T.astype(NPBF16)  # [D, S]
    a = xt.reshape(DK, 128, NSC, SC).transpose(2, 1, 0, 3)
    return np.ascontiguousarray(a)


def kernel(v, k, q, mask, wq_w, wq_b, wk_w, wk_b, wv_w, wv_b, dense_w, dense_b):
    global LAST_RESULT
    v = np.asarray(v, dtype=np.float32)
    k = np.asarray(k, dtype=np.float32)
    q = np.asarray(q, dtype=np.float32)
    mask = np.asarray(mask, dtype=np.float32)
    wq_w = np.asarray(wq_w, dtype=np.float32)
    wk_w = np.asarray(wk_w, dtype=np.float32)
    wv_w = np.asarray(wv_w, dtype=np.float32)
    dense_w = np.asarray(dense_w, dtype=np.float32)
    wq_b = np.asarray(wq_b, dtype=np.float32)
    wk_b = np.asarray(wk_b, dtype=np.float32)
    wv_b = np.asarray(wv_b, dtype=np.float32)
    dense_b = np.asarray(dense_b, dtype=np.float32)

    ktiles, mk_arr = _classify_mask(mask)
    key = (
        tuple(tuple((t, lo, tuple(tri)) for t, lo, tri in tl) for tl in ktiles),
        mk_arr.shape[1],
    )
    if key not in _CACHE:
        _CACHE[key] = _build(ktiles, mk_arr.shape[1])
    nc = _CACHE[key]

    # per-batch inputs (shared by the 4 cores of each batch)
    xq_b = [_xt_prep(q[b]) for b in range(B)]
    xk_b = [_xt_prep(k[b]) for b in range(B)]
    xv_b = [_xt_prep(v[b]) for b in range(B)]

    # per-group weights
    def wslice(w, g):
        ws = w[g * OG : (g + 1) * OG, :].T.astype(NPBF16)  # [D, OG]
        return np.ascontiguousarray(ws.reshape(DK, 128, OG).transpose(1, 0, 2))

    def bslice(b_, g):
        return np.ascontiguousarray(
            b_[g * OG : (g + 1) * OG].astype(np.float32).reshape(2, 128).T
        )

    wq_g = [wslice(wq_w, g) for g in range(GROUPS)]
    wk_g = [wslice(wk_w, g) for g in range(GROUPS)]
    wv_g = [wslice(wv_w, g) for g in range(GROUPS)]
    qb_g = [bslice(wq_b, g) for g in range(GROUPS)]
    kb_g = [bslice(wk_b, g) for g in range(GROUPS)]
    wd_g = []
    for g in range(GROUPS):
        ds = dense_w[:, g * OG : (g + 1) * OG].T.astype(NPBF16)  # [OG, D]
        wd_g.append(np.ascontiguousarray(ds.reshape(2, 128, D_MODEL).transpose(1, 0, 2)))

    in_maps = []
    for c in range(N_CORES):
        b, g = c // GROUPS, c % GROUPS
        in_maps.append(
            {
                "xq": xq_b[b],
                "xk": xk_b[b],
                "xv": xv_b[b],
                "wq": wq_g[g],
                "wk": wk_g[g],
                "wv": wv_g[g],
                "wd": wd_g[g],
                "qb": qb_g[g],
                "kb": kb_g[g],
                "mk": mk_arr,
            }
        )

    kw = dict(trace=True, **TRACE_KW) if TRACE else {}
    res = run_bass_kernel_spmd(nc, in_maps, core_ids=list(range(N_CORES)), **kw)
    LAST_RESULT = res

    corr = dense_w @ wv_b + dense_b  # v-bias pushed through dense, + dense bias
    out = np.empty((B, S, D_MODEL), dtype=np.float32)
    for b in range(B):
        acc = np.zeros((S, D_MODEL), dtype=np.float32)
        for g in range(GROUPS):
            acc += res.results[b * GROUPS + g]["outp"]
        out[b] = acc + corr
    return out
